# revision 27
# baseline (speedup 1.0000x reference)
"""Trainium2 Bass kernel for im2col conv2d + bias + channel-pack.

Semantics (matches the reference):
    out[c, w] = sum_k enc_x[w, k] * weight[c, k] + bias[c],  flattened to [C*W].

Strategy:
  - Shard the window dimension W=1048576 across 8 cores (131072 windows each).
  - Host-side: transpose enc_x to [K, W] (so the contraction dim K=49 lands on
    SBUF partitions) and cast to fp16 (halves HBM traffic; PE accumulates fp32).
  - Device-side: stationary operand is a block-diagonal [2K, 2C] weight matrix,
    so each matmul computes TWO 512-window chunks at once and the output tile
    occupies 64 partitions (keeps the scalar-engine bias/copy off the critical
    path). Bias is fused into the PSUM->SBUF copy via activation(Identity, bias).
  - Memory-bound regime: per-core HBM traffic = 12.8 MB in + 16.8 MB out.
"""

import os

import numpy as np

K = 49
C = 32
WINDOWS_NB = 1048576
N_CORES = 8
W_CORE = WINDOWS_NB // N_CORES  # 131072

# Device tiling parameters (full-size problem).
F = 8192  # windows per half input tile  (x_tile is [2K, F], covers 2F windows)
GROUP = 2048  # psum tile free dim (4 MM pairs of 512)
NMM = 512  # matmul moving free dim (one PSUM bank of fp32)

_PROGRAM_CACHE: dict = {}
LAST_RESULT = None  # BassKernelResults of the most recent run (for test harness)


def build_program(w_core=W_CORE, f=F, group=GROUP, nmm=NMM):
    import concourse.tile as tile
    from concourse import bacc, mybir

    pair = 2 * nmm  # windows-per-half covered by one concurrent MM pair
    assert w_core % (2 * f) == 0 and f % (4 * pair) == 0 and group == 4 * nmm
    n_outer = w_core // (2 * f)
    npair = f // pair  # MM pairs per outer iteration

    nc = bacc.Bacc("TRN2", debug=False, num_devices=N_CORES)
    # Host-shuffled input: xt2[it, j, k, p*1024 + h*512 + t] = enc_x^T fp16
    # value for window w = (it*npair + p)*2048 + (2h+j)*512 + t. This makes
    # every DMA in the kernel a <=3-dim AP with large uniform strides.
    xt = nc.dram_tensor(
        "xt", [w_core // (2 * f), 2, K, f], mybir.dt.float16, kind="ExternalInput"
    )
    # Block-diag weights duplicated into both 64-column halves of the PE
    # array: cols [64j..64j+31] = W for k-rows 0..48, cols [64j+32..64j+63]
    # = W for k-rows 49..97. Two matmuls on different column groups run
    # concurrently and fill all 128 PSUM partitions.
    w4 = nc.dram_tensor("w4", [2 * K, 4 * C], mybir.dt.float16, kind="ExternalInput")
    br = nc.dram_tensor("br", [4 * C, 1], mybir.dt.float32, kind="ExternalInput")
    # fp16 output (upcast on host): halves HBM write traffic, which is the
    # dominant cost in this memory-bound kernel.
    out = nc.dram_tensor("out", [C, w_core], mybir.dt.float16, kind="ExternalOutput")

    with tile.TileContext(nc) as tc:
        with tc.tile_pool(name="const", bufs=1) as cpool, \
             tc.tile_pool(name="xin", bufs=3) as xpool, \
             tc.tile_pool(name="osb", bufs=3) as opool, \
             tc.tile_pool(name="ps", bufs=2, space="PSUM") as ppool:
            w_sb = cpool.tile([2 * K, 4 * C], mybir.dt.float16)
            nc.sync.dma_start(out=w_sb, in_=w4.ap())
            b_sb = cpool.tile([4 * C, 1], mybir.dt.float32)
            nc.sync.dma_start(out=b_sb, in_=br.ap())

            xt_ap = xt.ap()
            assert n_outer % 2 == 0
            # out element [c, w]; w = ((i2*2*npair + G)*4 + jj)*nmm + t
            out_r = out.ap().rearrange(
                "c (i2 G jj t) -> i2 jj c G t",
                i2=n_outer // 2, G=2 * npair, jj=4, t=nmm,
            )

            o_tile = None
            for it in range(n_outer):
                u = it % 2
                x_tile = xpool.tile([2 * K, f], mybir.dt.float16)
                # Input rides two independent descriptor generators in
                # parallel: half0 on the scalar HWDGE ring (48+1 row split so
                # descriptors fan over all 16 engines: HWDGE uses the largest
                # engine count dividing the outer dim, and 49 -> only 7),
                # half1 on the gpsimd SWDGE path (partition-port spray, no
                # split needed). Doubles input instruction pacing.
                if it == 0:
                    # Fast ramp: the sync ring has no stores yet and SWDGE is
                    # slow to warm up (Q7 startup), so the first tile loads
                    # over both HWDGE rings in parallel.
                    nc.sync.dma_start(out=x_tile[0:48, :], in_=xt_ap[it, 0, 0:48])
                    nc.gpsimd.dma_start(out=x_tile[48:K, :], in_=xt_ap[it, 0, 48:K])
                    nc.scalar.dma_start(out=x_tile[K:K + 48, :], in_=xt_ap[it, 1, 0:48])
                    nc.gpsimd.dma_start(out=x_tile[K + 48:2 * K, :], in_=xt_ap[it, 1, 48:K])
                else:
                    nc.scalar.dma_start(out=x_tile[0:48, :], in_=xt_ap[it, 0, 0:48])
                    nc.gpsimd.dma_start(out=x_tile[48:K, :], in_=xt_ap[it, 0, 48:K])
                    nc.gpsimd.dma_start(out=x_tile[K:2 * K, :], in_=xt_ap[it, 1])
                if u == 0:
                    # One output tile spans TWO outer iterations so each store
                    # moves 512 KB: fewer DMA instructions on the store ring
                    # means fewer per-instruction completion stalls.
                    o_tile = opool.tile([4 * C, f], mybir.dt.float16)
                for q in range(npair // 4):
                    ps = ppool.tile([4 * C, group], mybir.dt.float32)
                    for r in range(4):
                        p = 4 * q + r
                        # concurrent MM pair on PE column groups 0-1 / 2-3
                        nc.tensor.matmul(
                            ps[0:2 * C, r * nmm:(r + 1) * nmm],
                            w_sb[:, 0:2 * C],
                            x_tile[:, p * pair:p * pair + nmm],
                            start=True,
                            stop=True,
                            tile_position=(0, 0),
                        )
                        nc.tensor.matmul(
                            ps[2 * C:4 * C, r * nmm:(r + 1) * nmm],
                            w_sb[:, 2 * C:4 * C],
                            x_tile[:, p * pair + nmm:(p + 1) * pair],
                            start=True,
                            stop=True,
                            tile_position=(0, 2 * C),
                        )
                    nc.scalar.activation(
                        o_tile[:, u * (f // 2) + q * group:u * (f // 2) + (q + 1) * group],
                        ps,
                        mybir.ActivationFunctionType.Identity,
                        bias=b_sb,
                        scale=1.0,
                    )
                if u == 1:
                    # One DMA per 32-partition block: DRAM-side outer dim 32
                    # (c) spreads descriptors over all 16 engines. Stores ride
                    # the sync HWDGE ring (higher queue priority than the
                    # scalar ring): they are throttled by compute anyway, so
                    # they preempt the input stream briefly instead of being
                    # starved by it.
                    for jj in range(4):
                        nc.sync.dma_start(
                            out=out_r[it // 2, jj],
                            in_=o_tile[jj * C:(jj + 1) * C, :].rearrange(
                                "c (G t) -> c G t", t=nmm
                            ),
                        )
    nc.compile()
    return nc


def _get_program():
    key = (W_CORE, F, GROUP, NMM)
    if key not in _PROGRAM_CACHE:
        _PROGRAM_CACHE[key] = build_program()
    return _PROGRAM_CACHE[key]


def shuffle_shard(x16t, f):
    """[K, w_core] fp16 -> [n_outer, 2, K, f] with the jj-interleaved window
    order the kernel's 3-dim store APs assume:
    xt2[it, j, k, p*1024 + h*512 + t] = x16t[k, ((it*npair+p)*4 + 2h+j)*512 + t]
    """
    w_core = x16t.shape[1]
    n_outer = w_core // (2 * f)
    npair = f // 1024
    x5 = x16t.reshape(K, n_outer, npair, 4, 512)
    parts = []
    for j in range(2):
        sel = x5[:, :, :, [j, j + 2], :]          # [K, n_outer, npair, 2, 512]
        parts.append(sel.transpose(1, 0, 2, 3, 4).reshape(n_outer, 1, K, f))
    return np.ascontiguousarray(np.concatenate(parts, axis=1))


def prepare_inputs(enc_x, weight, bias, f=F):
    """Host-side prep: per-core shuffled fp16 shards + block-diag weights."""
    enc_x = np.asarray(enc_x, dtype=np.float32)
    weight = np.asarray(weight, dtype=np.float32)
    bias = np.asarray(bias, dtype=np.float32)

    wflat = weight.reshape(C, K)
    wt16 = wflat.T.astype(np.float16)
    w4 = np.zeros((2 * K, 4 * C), dtype=np.float16)
    for j in range(2):
        w4[0:K, 2 * j * C:(2 * j + 1) * C] = wt16
        w4[K:2 * K, (2 * j + 1) * C:(2 * j + 2) * C] = wt16
    br = np.tile(bias, 4)[:, None].astype(np.float32)

    x16 = enc_x.astype(np.float16)
    shards = [
        shuffle_shard(np.ascontiguousarray(x16[i * W_CORE:(i + 1) * W_CORE].T), f)
        for i in range(N_CORES)
    ]
    return shards, w4, br


def kernel(enc_x, weight, bias, windows_nb=None):
    global LAST_RESULT
    from concourse import bass_utils

    shards, w4, br = prepare_inputs(enc_x, weight, bias)
    nc = _get_program()
    in_maps = [{"xt": shards[i], "w4": w4, "br": br} for i in range(N_CORES)]
    trace = bool(int(os.environ.get("BASS_KERNEL_TRACE", "0")))
    tmpdir = os.environ.get("BASS_KERNEL_TMPDIR") or None
    res = bass_utils.run_bass_kernel_spmd(
        nc, in_maps, core_ids=list(range(N_CORES)), trace=trace, tmpdir=tmpdir
    )
    LAST_RESULT = res
    outs = [res.results[i]["out"] for i in range(N_CORES)]
    return np.concatenate(outs, axis=1).astype(np.float32).reshape(-1)


# revision 30
# speedup vs baseline: 1.0214x; 1.0214x over previous
"""Trainium2 Bass kernel for im2col conv2d + bias + channel-pack.

Semantics (matches the reference):
    out[c, w] = sum_k enc_x[w, k] * weight[c, k] + bias[c],  flattened to [C*W].

Strategy:
  - Shard the window dimension W=1048576 across 8 cores (131072 windows each).
  - Host-side: transpose enc_x to [K, W] (so the contraction dim K=49 lands on
    SBUF partitions) and cast to fp16 (halves HBM traffic; PE accumulates fp32).
  - Device-side: stationary operand is a block-diagonal [2K, 2C] weight matrix,
    so each matmul computes TWO 512-window chunks at once and the output tile
    occupies 64 partitions (keeps the scalar-engine bias/copy off the critical
    path). Bias is fused into the PSUM->SBUF copy via activation(Identity, bias).
  - Memory-bound regime: per-core HBM traffic = 12.8 MB in + 16.8 MB out.
"""

import os

import numpy as np

K = 49
C = 32
WINDOWS_NB = 1048576
N_CORES = 8
W_CORE = WINDOWS_NB // N_CORES  # 131072

# Device tiling parameters (full-size problem).
F = 8192  # windows per half input tile  (x_tile is [2K, F], covers 2F windows)
GROUP = 2048  # psum tile free dim (4 MM pairs of 512)
NMM = 512  # matmul moving free dim (one PSUM bank of fp32)

_PROGRAM_CACHE: dict = {}
LAST_RESULT = None  # BassKernelResults of the most recent run (for test harness)


def build_program(w_core=W_CORE, f=F, group=GROUP, nmm=NMM):
    import concourse.tile as tile
    from concourse import bacc, mybir

    pair = 2 * nmm  # windows-per-half covered by one concurrent MM pair
    assert w_core % (2 * f) == 0 and f % (4 * pair) == 0 and group == 4 * nmm
    n_outer = w_core // (2 * f)
    npair = f // pair  # MM pairs per outer iteration

    nc = bacc.Bacc("TRN2", debug=False, num_devices=N_CORES)
    # Host-shuffled input: xt2[it, j, k, p*1024 + h*512 + t] = enc_x^T fp16
    # value for window w = (it*npair + p)*2048 + (2h+j)*512 + t. This makes
    # every DMA in the kernel a <=3-dim AP with large uniform strides.
    xt = nc.dram_tensor(
        "xt", [w_core // (2 * f), 2, K, f], mybir.dt.float16, kind="ExternalInput"
    )
    # Block-diag weights duplicated into both 64-column halves of the PE
    # array: cols [64j..64j+31] = W for k-rows 0..48, cols [64j+32..64j+63]
    # = W for k-rows 49..97. Two matmuls on different column groups run
    # concurrently and fill all 128 PSUM partitions.
    w4 = nc.dram_tensor("w4", [2 * K, 4 * C], mybir.dt.float16, kind="ExternalInput")
    br = nc.dram_tensor("br", [4 * C, 1], mybir.dt.float32, kind="ExternalInput")
    # fp16 output (upcast on host): halves HBM write traffic, which is the
    # dominant cost in this memory-bound kernel.
    out = nc.dram_tensor("out", [C, w_core], mybir.dt.float16, kind="ExternalOutput")

    with tile.TileContext(nc) as tc:
        with tc.tile_pool(name="const", bufs=1) as cpool, \
             tc.tile_pool(name="xin", bufs=3) as xpool, \
             tc.tile_pool(name="osb", bufs=3) as opool, \
             tc.tile_pool(name="ps", bufs=2, space="PSUM") as ppool:
            w_sb = cpool.tile([2 * K, 4 * C], mybir.dt.float16)
            nc.sync.dma_start(out=w_sb, in_=w4.ap())
            b_sb = cpool.tile([4 * C, 1], mybir.dt.float32)
            nc.sync.dma_start(out=b_sb, in_=br.ap())

            xt_ap = xt.ap()
            assert n_outer % 2 == 0
            # out element [c, w]; w = jj*(w_core/4) + (i2*2*npair + G)*nmm + t:
            # each jj partition-block owns a quarter of the window range, so
            # every store is a fully contiguous 2-dim [32, 4*npair*nmm] AP.
            out_r = out.ap().rearrange(
                "c (jj i2 s) -> i2 jj c s",
                jj=4, i2=n_outer // 2, s=2 * npair * nmm,
            )

            o_tile = None
            for it in range(n_outer):
                u = it % 2
                x_tile = xpool.tile([2 * K, f], mybir.dt.float16)
                # Input rides two independent descriptor generators in
                # parallel: half0 on the scalar HWDGE ring (48+1 row split so
                # descriptors fan over all 16 engines: HWDGE uses the largest
                # engine count dividing the outer dim, and 49 -> only 7),
                # half1 on the gpsimd SWDGE path (partition-port spray, no
                # split needed). Doubles input instruction pacing.
                if it == 0:
                    # Fast ramp: the sync ring has no stores yet and SWDGE is
                    # slow to warm up (Q7 startup), so the first tile loads
                    # over both HWDGE rings in parallel.
                    nc.sync.dma_start(out=x_tile[0:48, :], in_=xt_ap[it, 0, 0:48])
                    nc.gpsimd.dma_start(out=x_tile[48:K, :], in_=xt_ap[it, 0, 48:K])
                    nc.scalar.dma_start(out=x_tile[K:K + 48, :], in_=xt_ap[it, 1, 0:48])
                    nc.gpsimd.dma_start(out=x_tile[K + 48:2 * K, :], in_=xt_ap[it, 1, 48:K])
                else:
                    nc.scalar.dma_start(out=x_tile[0:48, :], in_=xt_ap[it, 0, 0:48])
                    nc.gpsimd.dma_start(out=x_tile[48:K, :], in_=xt_ap[it, 0, 48:K])
                    nc.gpsimd.dma_start(out=x_tile[K:2 * K, :], in_=xt_ap[it, 1])
                if u == 0:
                    # One output tile spans TWO outer iterations so each store
                    # moves 512 KB: fewer DMA instructions on the store ring
                    # means fewer per-instruction completion stalls.
                    o_tile = opool.tile([4 * C, f], mybir.dt.float16)
                for q in range(npair // 4):
                    ps = ppool.tile([4 * C, group], mybir.dt.float32)
                    for r in range(4):
                        p = 4 * q + r
                        # concurrent MM pair on PE column groups 0-1 / 2-3
                        nc.tensor.matmul(
                            ps[0:2 * C, r * nmm:(r + 1) * nmm],
                            w_sb[:, 0:2 * C],
                            x_tile[:, p * pair:p * pair + nmm],
                            start=True,
                            stop=True,
                            tile_position=(0, 0),
                        )
                        nc.tensor.matmul(
                            ps[2 * C:4 * C, r * nmm:(r + 1) * nmm],
                            w_sb[:, 2 * C:4 * C],
                            x_tile[:, p * pair + nmm:(p + 1) * pair],
                            start=True,
                            stop=True,
                            tile_position=(0, 2 * C),
                        )
                    nc.scalar.activation(
                        o_tile[:, u * (f // 2) + q * group:u * (f // 2) + (q + 1) * group],
                        ps,
                        mybir.ActivationFunctionType.Identity,
                        bias=b_sb,
                        scale=1.0,
                    )
                if u == 1:
                    # One DMA per 32-partition block: DRAM-side outer dim 32
                    # (c) spreads descriptors over all 16 engines. Stores ride
                    # the sync HWDGE ring (higher queue priority than the
                    # scalar ring): they are throttled by compute anyway, so
                    # they preempt the input stream briefly instead of being
                    # starved by it.
                    for jj in range(4):
                        nc.sync.dma_start(
                            out=out_r[it // 2, jj],
                            in_=o_tile[jj * C:(jj + 1) * C, :],
                        )
    nc.compile()
    return nc


def _get_program():
    key = (W_CORE, F, GROUP, NMM)
    if key not in _PROGRAM_CACHE:
        _PROGRAM_CACHE[key] = build_program()
    return _PROGRAM_CACHE[key]


def shuffle_shard(x16t, f):
    """[K, w_core] fp16 -> [n_outer, 2, K, f] with the window order the
    kernel's contiguous store APs assume: partition block jj = 2h+j owns the
    jj-th quarter of the core's window range, i.e.
    xt2[it, j, k, p*1024 + h*512 + t]
        = x16t[k, (2h+j)*(w_core//4) + (it*npair + p)*512 + t]
    """
    w_core = x16t.shape[1]
    n_outer = w_core // (2 * f)
    npair = f // 1024
    x4 = x16t.reshape(K, 4, n_outer, npair, 512)  # [k, jj, it, p, t]
    parts = []
    for j in range(2):
        sel = x4[:, [j, j + 2], :, :, :]          # [K, 2(h), n_outer, npair, 512]
        parts.append(sel.transpose(2, 0, 3, 1, 4).reshape(n_outer, 1, K, f))
    return np.ascontiguousarray(np.concatenate(parts, axis=1))


def prepare_inputs(enc_x, weight, bias, f=F):
    """Host-side prep: per-core shuffled fp16 shards + block-diag weights."""
    enc_x = np.asarray(enc_x, dtype=np.float32)
    weight = np.asarray(weight, dtype=np.float32)
    bias = np.asarray(bias, dtype=np.float32)

    wflat = weight.reshape(C, K)
    wt16 = wflat.T.astype(np.float16)
    w4 = np.zeros((2 * K, 4 * C), dtype=np.float16)
    for j in range(2):
        w4[0:K, 2 * j * C:(2 * j + 1) * C] = wt16
        w4[K:2 * K, (2 * j + 1) * C:(2 * j + 2) * C] = wt16
    br = np.tile(bias, 4)[:, None].astype(np.float32)

    x16 = enc_x.astype(np.float16)
    shards = [
        shuffle_shard(np.ascontiguousarray(x16[i * W_CORE:(i + 1) * W_CORE].T), f)
        for i in range(N_CORES)
    ]
    return shards, w4, br


def kernel(enc_x, weight, bias, windows_nb=None):
    global LAST_RESULT
    from concourse import bass_utils

    shards, w4, br = prepare_inputs(enc_x, weight, bias)
    nc = _get_program()
    in_maps = [{"xt": shards[i], "w4": w4, "br": br} for i in range(N_CORES)]
    trace = bool(int(os.environ.get("BASS_KERNEL_TRACE", "0")))
    tmpdir = os.environ.get("BASS_KERNEL_TMPDIR") or None
    res = bass_utils.run_bass_kernel_spmd(
        nc, in_maps, core_ids=list(range(N_CORES)), trace=trace, tmpdir=tmpdir
    )
    LAST_RESULT = res
    outs = [res.results[i]["out"] for i in range(N_CORES)]
    return np.concatenate(outs, axis=1).astype(np.float32).reshape(-1)
